# revision 26
# baseline (speedup 1.0000x reference)
"""Trainium2 Bass kernel for PVT-style cross-batch sparse attention.

Reference computation (per batch element b, with partner p = (b+4)%8):
    q  = x[b] @ Wq.T                                  [4096, 128]
    xr = conv_stride4(x[p]) + sr_b  -> layernorm      [256, 128]
    kv = xr @ Wkv.T ; k, v heads (2 heads, hd=64)
    out = softmax(q k^T / sqrt(hd)) v ; proj Wp + bp  [4096, 128]

Sharding: one batch element per NeuronCore (8 cores). Each core gets its
own x (for q) and its partner's x (for k/v). No collectives.

Key structure (all layouts channel-transposed, [C, T]):
  - q-projection is fused into the score matmuls: scores = (k Wq s) x^T,
    with M = k_h Wq_h precomputed per head ([keys, C], tiny), so x feeds
    the score matmuls directly and the q tensor never exists.
  - scores run as fp8e4 DoubleRow matmuls (K=128 split into 2x64
    channel pairs); M is scaled by 2^SM host-side and the exp descales.
  - av and the softmax denominator run as bf16 DoubleRow matmuls
    (K=256 keys = 2x128 interleaved): one matmul per head for av, one
    with an all-ones lhsT for dn, whose output rows are the denominator
    pre-broadcast. A single vector divide (av / dn) replaces
    reciprocal+multiply.
  - exp work is split across ACT (native Exp) and DVE/GPSIMD via
    pow(E, s) with E = e^(2^-SM), to balance engine load.
  - LN weight/bias, attention scale and all other affine constants are
    folded host-side (ln_b exactly folds into the output bias).
"""

import numpy as np
import ml_dtypes

import concourse.bass as bass
import concourse.tile as tile
from concourse import mybir
from concourse.bass_utils import run_bass_kernel_spmd


# ---------------------------------------------------------------------------
# The tail drain TileContext emits waits on every processor's final tick in
# ONE instruction, which exceeds this toolchain's per-instruction sync-wait
# budget. Split it: emit one single-wait drain per active proc first (the
# wait-clock elision then leaves the final drain with nothing to wait on).
from bass_rust import ScopedClock, VectorClock
from concourse.tile_scheduler import N_PROCS


def _split_drain_and_barrier(self, tick_clock, wait_clock):
    full = tick_clock.global_clock
    for p in range(N_PROCS):
        t = full[p]
        if t <= 0:
            continue
        ticks = [0] * N_PROCS
        ticks[p] = t
        d = self.nc.sync.drain()
        wait_clock.add_sem_waits(d.ins, ScopedClock({None: VectorClock(ticks)}))
    self.nc.sync.drain()

    self.nc.all_engine_barrier()
    assert self.sems is not None
    popped = self.nc._tile_sem_poison_stack.pop()
    assert popped is self._sem_poison
    self.nc.clear_and_free_semaphores(list(self.sems.allocated().values()))
    self.nc.all_engine_barrier()


tile.TileContext._drain_and_barrier = _split_drain_and_barrier

BF16 = mybir.dt.bfloat16
F32 = mybir.dt.float32
FP8 = mybir.dt.float8e4
DR = mybir.MatmulPerfMode.DoubleRow

B, T, C = 8, 4096, 128
NH, HD = 2, 64
SR = 4
H = W = 64
OH = OW = 16
NK = OH * OW          # 256 reduced tokens
SCALE = HD ** -0.5
SM = 9                # score matmul scale exponent (fp8 range usage)
NG = 8                # granules of 512 query tokens
GT = T // NG          # 512

# exp must run on ACT: GPSIMD cannot access PSUM and the DVE engine does
# not implement pow (neuronx-cc rejects it at codegen)
def exp_engine(g, jt):
    return 'a'


# otg copy engine per otg tile (4 granules each): single engine per tile
# so the store DMA carries one wait. DVE, since ACT is exp-saturated.
OTG_ENGINE = ['v', 'v']

# blob column offsets (bf16 columns)
O_XKV = 0
O_XQ8 = O_XKV + T          # fp8 DR-layout x for scores: [64, (2, T)] bytes
O_WQ = O_XQ8 + T
O_WKV = O_WQ + C
O_WP = O_WKV + 2 * C
O_SRW = O_WP + C
O_ONES = O_SRW + 16 * C    # ones [128, 128]
O_INV = O_ONES + C         # col 0 = 1/128
O_VECS = O_INV + 8         # 4 f32: srb, eps, E=exp(2^-SM), pad
NBLOB = O_VECS + 8


def build_nc(out_bf16: bool = True, niter: int = 1,
             store_last_only: bool = False):
    nc = bass.Bass()

    blob = nc.declare_dram_parameter("blob", [C, NBLOB], BF16, isOutput=False)
    out_dt = BF16 if out_bf16 else F32
    out = nc.declare_dram_parameter("out", [T, C], out_dt, isOutput=True)

    with tile.TileContext(nc) as tc:
        const = tc.alloc_tile_pool(name="const", bufs=1)
        work = tc.alloc_tile_pool(name="work", bufs=2)
        psum = tc.alloc_tile_pool(name="psum", bufs=1, space="PSUM")

        blob_sb = const.tile([128, NBLOB], BF16)
        # split the load into 4 DMAs on separate queues / issuing engines
        nc.sync.dma_start(out=blob_sb[:, O_XKV:O_XKV + T // 2],
                          in_=blob[:, O_XKV:O_XKV + T // 2])
        nc.sync.dma_start(out=blob_sb[:, O_XKV + T // 2:O_XKV + T],
                          in_=blob[:, O_XKV + T // 2:O_XKV + T])
        nc.scalar.dma_start(out=blob_sb[:, O_WQ:NBLOB], in_=blob[:, O_WQ:NBLOB])
        nc.gpsimd.dma_start(out=blob_sb[:, O_XQ8:O_XQ8 + T],
                            in_=blob[:, O_XQ8:O_XQ8 + T])

        xkvT = blob_sb[:, O_XKV:O_XKV + T]
        xq8 = blob_sb[:, O_XQ8:O_XQ8 + T].bitcast(FP8).rearrange(
            "c (i t) -> c i t", i=2)            # [128, 2, 4096], rows 0:64
        wq_sc = blob_sb[:, O_WQ:O_WQ + C]        # Wq*scale*2^SM, [qc, C]
        wkv_sb = blob_sb[:, O_WKV:O_WKV + 2 * C]
        wp_sb = blob_sb[:, O_WP:O_WP + C]
        srw3 = blob_sb[:, O_SRW:O_SRW + 16 * C].rearrange("c (t o) -> c t o", t=16)
        ones128 = blob_sb[:, O_ONES:O_ONES + C]
        ones64 = ones128[:, 0:64]
        ones_row = ones128[0:1, :]
        inv128_col = blob_sb[:, O_INV:O_INV + 1]
        vecs_f = blob_sb[:, O_VECS:O_VECS + 8].bitcast(F32)
        srb_sb = vecs_f[:, 0:1]
        eps_t = vecs_f[0:1, 1:2]
        E_col = vecs_f[:, 2:3]                   # e^(2^-SM)

        # dummy engine reads of the blob: advance observed clocks past the
        # load DMAs so early ops need no extra waits
        vtouch = const.tile([1, 1], F32)
        nc.vector.tensor_copy(out=vtouch, in_=vecs_f[0:1, 0:1])
        vtouch2 = const.tile([1, 1], F32)
        nc.scalar.copy(out=vtouch2, in_=vecs_f[0:1, 0:1])
        # PE touches: observe the xkv and xq8 load DMAs so conv / score
        # matmuls need no DMA-queue waits of their own
        xq_bf = blob_sb[:, O_XQ8:O_XQ8 + T]
        xt_ps = psum.tile([1, 1], F32, tag="big", bufs=2)
        nc.tensor.matmul(xt_ps, lhsT=xkvT[0:1, 0:1], rhs=xkvT[0:1, 0:1],
                         start=True, stop=True)
        xt_ps2 = psum.tile([1, 1], F32, tag="big", bufs=2)
        nc.tensor.matmul(xt_ps2, lhsT=xq_bf[0:1, 0:1], rhs=xq_bf[0:1, 0:1],
                         start=True, stop=True)

        acts = tc.alloc_tile_pool(name="acts", bufs=1)
        lnT = acts.tile([128, NK], BF16)
        kT = acts.tile([128, NK], BF16)
        # M8 [64, (h, jt, i, k)]: score lhsT for (h, jt) = M8[0:64, h, jt]
        M8 = acts.tile([128, 2, 2, 2, 128], FP8)
        v_dr = acts.tile([128, 2, 2, 64], BF16)  # [keypos, (h, jt, chan)]

        # out DRAM view: token = g*2048 + s*256 + 2m + par
        out9 = out[:].rearrange("(g s m j) c -> g m s (j c)", s=8, m=128, j=2)

        for its in range(niter):
            do_store = (not store_last_only) or (its == niter - 1)
            P = f"i{its}_"

            # ---------------- conv (spatial reduction) ----------------
            x5 = xkvT.rearrange("c (oh kh ow kw) -> c oh kh ow kw",
                                oh=OH, kh=4, ow=OW, kw=4)
            conv_ps = psum.tile([128, NK], F32, tag="w512", bufs=2,
                                name=P + "conv")
            for ohh in range(2):
                for tap in range(16):
                    kh, kw = tap // 4, tap % 4
                    nc.tensor.matmul(
                        conv_ps[:, ohh * 128:(ohh + 1) * 128],
                        lhsT=srw3[:, tap, :],
                        rhs=x5[:, ohh * 8:(ohh + 1) * 8, kh, :, kw],
                        start=(tap == 0), stop=(tap == 15))

            # ---------------- LayerNorm over channels ----------------
            convT = work.tile([128, NK], BF16, tag="convT", name=P + "convT")
            nc.vector.tensor_scalar_add(out=convT, in0=conv_ps, scalar1=srb_sb)
            sq = work.tile([128, NK], BF16, tag="sq", name=P + "sq")
            nc.vector.tensor_tensor(out=sq, in0=convT, in1=convT,
                                    op=mybir.AluOpType.mult)
            st_ps = psum.tile([1, 2 * NK], F32, tag="w512", bufs=2,
                              name=P + "st")
            nc.tensor.matmul(st_ps[:, 0:NK], lhsT=inv128_col, rhs=convT,
                             start=True, stop=True)
            nc.tensor.matmul(st_ps[:, NK:2 * NK], lhsT=inv128_col, rhs=sq,
                             start=True, stop=True)
            # all st_ps readers on ACT so its psum slot has a single
            # consumer engine
            mu2 = work.tile([1, NK], F32, tag="mu2", name=P + "mu2")
            nc.scalar.square(out=mu2, in_=st_ps[:, 0:NK])
            ex2 = work.tile([1, NK], F32, tag="ex2", name=P + "ex2")
            nc.scalar.mul(out=ex2, in_=st_ps[:, NK:2 * NK], mul=1.0)
            brow = work.tile([1, 2 * NK], BF16, tag="brow", name=P + "brow")
            nc.scalar.copy(out=brow[:, 0:NK], in_=st_ps[:, 0:NK])
            var = work.tile([1, NK], F32, tag="var", name=P + "var")
            nc.vector.tensor_sub(out=var, in0=ex2, in1=mu2)
            # rstd = exp(-0.5 * ln(var + eps)); Ln+Exp share one ACT table set
            lnv = work.tile([1, NK], F32, tag="lnv", name=P + "lnv")
            nc.scalar.activation(out=lnv, in_=var,
                                 func=mybir.ActivationFunctionType.Ln,
                                 bias=eps_t, scale=1.0)
            nc.scalar.activation(out=brow[:, NK:2 * NK], in_=lnv,
                                 func=mybir.ActivationFunctionType.Exp,
                                 scale=-0.5)
            bc_ps = psum.tile([128, 2 * NK], F32, tag="w512", bufs=2,
                              name=P + "bc")
            nc.tensor.matmul(bc_ps, lhsT=ones_row, rhs=brow, start=True,
                             stop=True)
            t1 = work.tile([128, NK], BF16, tag="t1", name=P + "t1")
            nc.vector.tensor_sub(out=t1, in0=convT, in1=bc_ps[:, 0:NK])
            nc.vector.tensor_mul(out=lnT, in0=t1, in1=bc_ps[:, NK:2 * NK])

            # ---------------- k, v, M = k_h Wq_h ----------------
            # all psum->sbuf copies here run on DVE so every head matmul's
            # slot-recycle dep merges into one DVE wait. M8 copies are
            # emitted LAST so the first score matmul's M8 wait also covers
            # kT/v_dr.
            kv_ps = psum.tile([128, NK], F32, tag="w512", bufs=2,
                              name=P + "kv")
            # guard: kv's slot WAR is on ACT (st_ps readers); absorb it in a
            # dummy so the real matmul only waits on DVE (lnT)
            nc.tensor.matmul(kv_ps[0:1, 0:1], lhsT=brow[0:1, 0:1],
                             rhs=brow[0:1, 0:1], start=True, stop=True,
                             skip_group_check=True)
            nc.tensor.matmul(kv_ps, lhsT=wkv_sb[:, 0:C], rhs=lnT,
                             start=True, stop=True, skip_group_check=True)
            nc.vector.tensor_copy(out=kT, in_=kv_ps)

            for jt in range(2):
                v_ps = psum.tile([128, 128], F32, tag="w512", bufs=2,
                                 name=P + f"v_ps{jt}")
                nc.tensor.matmul(v_ps, lhsT=lnT[:, jt * 128:(jt + 1) * 128],
                                 rhs=wkv_sb[:, C:2 * C], start=True, stop=True)
                nc.vector.tensor_copy(
                    out=v_dr[:, :, jt, :],
                    in_=v_ps.rearrange("p (h c) -> p h c", h=2))

            # M_h^T [C, keys] = Wq_h^T @ kT_h ; cast to fp8 DR layout
            for h in range(2):
                m_ps = psum.tile([128, NK], F32, tag="w512", bufs=2,
                                 name=P + f"m_ps{h}")
                nc.tensor.matmul(m_ps, lhsT=wq_sc[h * 64:(h + 1) * 64, :],
                                 rhs=kT[h * 64:(h + 1) * 64, :],
                                 start=True, stop=True)
                for i in range(2):
                    nc.vector.tensor_copy(
                        out=M8[0:64, h, :, i, :],
                        in_=m_ps[i * 64:(i + 1) * 64, :].rearrange(
                            "c (jt k) -> c jt k", jt=2))

            # ------- attention + projection (granule-pipelined) -------
            sps_by_g = {}
            pt_by_g = {}
            outT_by_g = {}
            pj_by_g = {}
            otg_box = [None]

            def emit_scores(g):
                t0 = g * GT
                tiles = []
                for jt in range(2):
                    sps = psum.tile([128, 1024], F32, tag="big", bufs=2,
                                    name=P + f"sps_{g}_{jt}")
                    for h in range(2):
                        nc.tensor.matmul(
                            sps[:, h * GT:(h + 1) * GT],
                            lhsT=M8[0:64, h, jt],
                            rhs=xq8[0:64, :, t0:t0 + GT],
                            start=True, stop=True, perf_mode=DR)
                    tiles.append(sps)
                sps_by_g[g] = tiles

            def emit_exp(g):
                tiles = []
                for jt in range(2):
                    eng = exp_engine(g, jt)
                    tag = "pt0" if jt == 0 else ("pt1v" if eng == 'v' else "pt1a")
                    pt = work.tile([128, 2, GT], BF16, tag=tag, bufs=2,
                                   name=P + f"pt_{g}_{jt}")
                    sps = sps_by_g[g][jt]
                    if eng == 'a':
                        nc.scalar.activation(
                            out=pt, in_=sps,
                            func=mybir.ActivationFunctionType.Exp,
                            scale=float(2.0 ** -SM))
                    else:
                        e_in = E_col.broadcast_to((128, 2, GT))
                        nc.vector.tensor_tensor(
                            out=pt, in0=e_in, in1=sps,
                            op=mybir.AluOpType.pow)
                    tiles.append(pt)
                pt_by_g[g] = tiles

            def emit_avdn(g, tail=False):
                pt0, pt1 = pt_by_g[g]
                pts = [pt0, pt1]
                av = psum.tile([128, GT], F32, tag="avdn", bufs=2,
                               name=P + f"av_{g}")
                dn = psum.tile([128, GT], F32, tag="avdn", bufs=2,
                               name=P + f"dn_{g}")
                if tail and exp_engine(g, 1) == 'a':
                    # no scores(g+1) follow to absorb the ACT wait for pt1,
                    # so the first av matmul would carry 2 waits (ACT data +
                    # DVE slot-recycle). Absorb the DVE one in a guard.
                    gate = outT_by_g[g - 1]
                    nc.tensor.matmul(av[0:1, 0:1], lhsT=gate[0:1, 0:1],
                                     rhs=gate[0:1, 0:1], start=True,
                                     stop=True, skip_group_check=True)
                # av and dn phase-shifted so opposite PE column groups can
                # stay concurrently busy on hardware; jt1 first so the
                # slot-recycle dep merges with the pt1 data dep
                for idx in range(2):
                    ah, dh = idx, 1 - idx
                    for jt in (1, 0):
                        nc.tensor.matmul(
                            av[ah * 64:(ah + 1) * 64, :],
                            lhsT=v_dr[:, ah, jt, :], rhs=pts[jt][:, ah, :],
                            start=(jt == 1), stop=(jt == 0),
                            tile_position=(0, ah * 64),
                            skip_group_check=tail and idx == 0)
                        nc.tensor.matmul(
                            dn[dh * 64:(dh + 1) * 64, :],
                            lhsT=ones64, rhs=pts[jt][:, dh, :],
                            start=(jt == 1), stop=(jt == 0),
                            tile_position=(0, dh * 64))
                # vector ops may read only one PSUM operand: reciprocal the
                # denominator to SBUF, then multiply
                rec = work.tile([128, GT], F32, tag="rec", bufs=2,
                                name=P + f"rec_{g}")
                nc.vector.reciprocal(out=rec, in_=dn)
                outT = work.tile([128, GT], BF16, tag="outT", bufs=3,
                                 name=P + f"outT_{g}")
                nc.vector.tensor_mul(out=outT, in0=av, in1=rec)
                outT_by_g[g] = outT

            def emit_proj(g):
                oT3 = outT_by_g[g].rearrange("c (h m j) -> c h j m", h=2, j=2)
                pj = psum.tile([128, GT], F32, tag="w512", bufs=2,
                               name=P + f"pj_{g}")
                for h256 in range(2):
                    for par in range(2):
                        idx = h256 * 2 + par
                        nc.tensor.matmul(
                            pj[:, idx * 128:(idx + 1) * 128],
                            lhsT=oT3[:, h256, par, :], rhs=wp_sb,
                            start=True, stop=True)
                pj_by_g[g] = pj

            def emit_otg(g):
                sc, half = g // 2, g % 2
                if g % 4 == 0:
                    otg_box[0] = work.tile([128, 8, 256], out_dt, tag="osb",
                                           bufs=2, name=P + f"otg_{g // 4}")
                otg = otg_box[0]
                s0 = (sc % 2) * 4 + half * 2
                pj4 = pj_by_g[g].rearrange("m (h j c) -> m h j c", h=2, j=2)
                dst = otg[:, s0:s0 + 2, :].rearrange("m h (j c) -> m h j c", j=2)
                if OTG_ENGINE[g // 4] == 'a':
                    nc.scalar.copy(out=dst, in_=pj4)
                else:
                    nc.vector.tensor_copy(out=dst, in_=pj4)
                del pj_by_g[g]
                if g % 4 == 3 and do_store:
                    nc.sync.dma_start(out=out9[g // 4], in_=otg)

            for g in range(NG):
                emit_scores(g)
                if g >= 1:
                    emit_avdn(g - 1)
                emit_exp(g)
                if g >= 2:
                    emit_proj(g - 2)
                    emit_otg(g - 2)
            emit_avdn(NG - 1, tail=True)
            emit_proj(NG - 2)
            emit_otg(NG - 2)
            emit_proj(NG - 1)
            emit_otg(NG - 1)

        psum.release()
        acts.release()
        work.release()
        const.release()

    _strip_self_waits(nc)
    return nc


def _strip_self_waits(nc):
    """Drop waits on an instruction's own engine semaphore.

    Compute engines are in-order and bump their semaphore at completion, so
    a wait on the engine's own sem (emitted for psum-slot WAR recycling) is
    always satisfied by program order. The NEFF codegen allows only one
    sync wait per instruction, so these vacuous waits must go.
    """
    strip = {mybir.EngineType.PE: "PE", mybir.EngineType.DVE: "DVE",
             mybir.EngineType.Activation: "Activation",
             mybir.EngineType.Pool: "Pool"}
    for f in nc.m.functions:
        for blk in f.blocks:
            for ins in blk.instructions:
                eng = strip.get(ins.engine)
                if eng is None:
                    continue
                si = ins.sync_info
                if si is None or not si.on_wait or len(si.on_wait) < 2:
                    continue
                kept = [w for w in si.on_wait
                        if w.ant_name.split("_")[0] != eng]
                if kept and len(kept) != len(si.on_wait):
                    si.on_wait = kept


_NC_CACHE = {}


def _get_nc(out_bf16=True):
    key = out_bf16
    if key not in _NC_CACHE:
        _NC_CACHE[key] = build_nc(out_bf16)
    return _NC_CACHE[key]


def make_in_maps(x, Wq, Wkv, sr_w, sr_b, ln_w, ln_b, Wp):
    bf = ml_dtypes.bfloat16
    e4 = ml_dtypes.float8_e4m3
    x = np.asarray(x, np.float32)
    ln_w = np.asarray(ln_w, np.float32)
    wq_sc = (np.asarray(Wq, np.float32) * (SCALE * 2.0 ** SM)).astype(bf)
    # fold ln_w into both kv projections (k bias drops out of softmax,
    # v bias folds into the output bias host-side)
    wkv_t = (np.asarray(Wkv, np.float32).T * ln_w[:, None]).astype(bf)
    wp_t = np.asarray(Wp, np.float32).T.astype(bf)
    srw = np.asarray(sr_w, np.float32).transpose(1, 2, 3, 0).reshape(C, 16 * C).astype(bf)
    vecs = np.stack([np.asarray(sr_b, np.float32),
                     np.full(C, 1e-5, np.float32),
                     np.full(C, np.exp(2.0 ** -SM), np.float32),
                     np.zeros(C, np.float32)], axis=1)
    vecs_bits = np.ascontiguousarray(vecs).view(np.uint16).view(bf)

    wpart = np.empty((C, NBLOB - O_WQ), bf)
    wpart[:, 0:C] = wq_sc
    wpart[:, O_WKV - O_WQ:O_WP - O_WQ] = wkv_t
    wpart[:, O_WP - O_WQ:O_SRW - O_WQ] = wp_t
    wpart[:, O_SRW - O_WQ:O_ONES - O_WQ] = srw
    wpart[:, O_ONES - O_WQ:O_INV - O_WQ] = np.ones((C, C), bf)
    inv = np.zeros((C, 8), np.float32)
    inv[:, 0] = 1.0 / 128.0
    wpart[:, O_INV - O_WQ:O_VECS - O_WQ] = inv.astype(bf)
    wpart[:, O_VECS - O_WQ:] = vecs_bits

    xT = np.ascontiguousarray(x.transpose(0, 2, 1)).astype(bf)  # [B, C, T]
    # fp8 DR layout for scores: xq8[p, i, t] = fp8(x[t, p + 64 i])
    x8 = np.ascontiguousarray(
        x.transpose(0, 2, 1).reshape(B, 2, 64, T).transpose(0, 2, 1, 3)
    ).astype(e4)  # [B, 64, 2, T]
    in_maps = []
    for i in range(8):
        p = (i + 4) % 8
        blob = np.zeros((C, NBLOB), bf)
        blob[:, O_XKV:O_XKV + T] = xT[p]
        xq8_region = blob[:, O_XQ8:O_XQ8 + T].view(np.uint16).view(np.uint8)
        xq8_region[0:64, :] = x8[i].reshape(64, 2 * T).view(np.uint8)
        blob[:, O_WQ:] = wpart
        in_maps.append({"blob": blob})
    return in_maps


def kernel(x, Wq, Wkv, sr_w, sr_b, ln_w, ln_b, Wp, bp, H, W):
    assert int(H) == 64 and int(W) == 64
    in_maps = make_in_maps(x, Wq, Wkv, sr_w, sr_b, ln_w, ln_b, Wp)
    nc = _get_nc(out_bf16=True)
    res = run_bass_kernel_spmd(nc, in_maps, list(range(8)))
    outs = res.results
    r = np.stack([np.asarray(outs[i]["out"], np.float32) for i in range(8)])
    # ln_b propagates exactly to a constant output row: bp + Wp (Wkv_v ln_b)
    bv = np.asarray(Wkv, np.float32)[C:] @ np.asarray(ln_b, np.float32)
    bias = np.asarray(bp, np.float32) + np.asarray(Wp, np.float32) @ bv
    return r + bias[None, None, :]
